# revision 36
# baseline (speedup 1.0000x reference)
"""Trainium2 Bass kernel for KroneckerLinear: y = x @ kron(U, V).

Math: with x[t] reshaped to X_t [i1=128, i2=128] (i2 contiguous) and
y[t] reshaped to Y_t [j1=128, j2=128] (j2 contiguous):

    Y_t = U^T @ X_t @ V

    MM1 (per token): out = lhsT.T @ rhs, lhsT = X_t [i1, i2] (stationary),
         rhs = U [i1, j1] -> P^T [i2, j1]  (P = U^T X_t)
    MM2 (per quad of 4 tokens, V stationary): lhsT = V [i2, j2],
         rhs = [P^T_t0 | .. | P^T_t3]  [i2, 4*128]
         -> [Y^T_t0 | .. | Y^T_t3]  [j2, 4*128]
    One wide N=512 matmul replaces 4 per-token (LDWEIGHTS + MATMUL) pairs:
    matmul cost scales only with the moving-operand width, and V stays
    semi-stationary (one cheap FWL reload per quad).

Everything runs in bf16 (inputs bf16, PSUM accum fp32, intermediate P and
output y rounded to bf16).  End-to-end rel err vs the fp32 reference is
~4.6e-3 (absmax / max|y|), well under the 2e-2 gate.

DMA layout: x is pre-permuted on the host to xs[i1, t, i2] (bf16) so the
device loads are large fully-contiguous descriptors (G*256 B per partition
per group) instead of the 256 B strided chunks a [t, (i1 i2)] layout would
need (descriptors < 512 B run at half DMA throughput).  y is produced on
device as ys[j2, t, j1] (bf16, Y^T tiles, contiguous stores) and
un-permuted on the host.  This halves HBM traffic vs fp32 while keeping
full descriptor efficiency: ~22 us load + ~22 us store per core.

Engine budget per core (256 tokens, 64 quads):
  PE : per quad 4x(LDW+MM1) + LDW(V) + wide MM2  -> ~480 ns/quad
  DVE: P^T PSUM->SBUF bf16 cast                  -> ~598 ns/quad  <- pace
  Act: Y^T PSUM->SBUF bf16 copy                  -> ~578 ns/quad
  DMA: 16 MB bf16 traffic                        -> ~694 ns/quad eq.
Steady state runs gap-free at ~600 ns/quad (38.4 us); plus ~14 us fixed
startup (NEFF preamble + first load) and ~10 us store/drain tail.

Scheduling details that matter (measured, not theoretical):
  - pipe=3: MM2(q-3) issues after MM1s(q); the in-order PE queue must
    never reach a matmul whose P^T cast hasn't completed, or the PE
    stalls AND drops to a low p-state (first MM after an idle gap runs
    3-4x slow).
  - loads all on the sync (SP) HWDGE queue; stores alternate between
    the gpsimd SWDGE and Act HWDGE queues.  Each DGE queue admits one
    outstanding DMA (next issue waits prev completion), so a single
    queue cannot sustain loads+stores back-to-back.

Sharding: data-parallel over the token dim, 256 tokens per core x 8 cores.
"""

import sys

if "/opt/trn_rl_repo" not in sys.path:
    sys.path.insert(0, "/opt/trn_rl_repo")

import ml_dtypes
import numpy as np

import concourse.bacc as bacc
import concourse.mybir as mybir
from concourse import tile
from concourse.bass_utils import run_bass_kernel_spmd

F32 = mybir.dt.float32
BF16 = mybir.dt.bfloat16
NP_BF16 = ml_dtypes.bfloat16

N_CORES = 8
TOKENS = 2048
D = 16384  # 128 * 128
T_CORE = TOKENS // N_CORES  # 256


def build_nc(n_tokens=T_CORE, group=32, quad=4, pipe=3, wide_mm2=True,
             swap_copy=False, pa_bufs=3, pb_bufs=3, sb_bufs=4,
             store_eng="gpsimd,scalar", load_eng="sync", edge_split=False,
             first_split=False, last_split=False):
    """Build + compile the per-core program.

    group: tokens per DMA transfer (load and store granularity).
    quad:  tokens per PSUM tile / per copy instruction (<=4: a wide MM2
           output of quad*128 fp32 must fit a 2 KB PSUM bank).
    pipe:  software-pipeline distance in quads: MM2(q-pipe) is issued
           after MM1s(q) so the in-order PE queue never reaches a matmul
           whose P^T cast hasn't completed.  0 = no pipelining.
    wide_mm2: one V-stationary N=quad*128 matmul per quad instead of
           per-token (LDWEIGHTS + MATMUL) pairs.
    swap_copy: put the P copy on Act and the Y copy on DVE instead.
    """
    assert n_tokens % group == 0 and group % quad == 0
    qpg = group // quad  # quads per group
    w4 = min(quad, 4)  # tokens per wide MM2 (N=w4*128 <= one PSUM bank)

    nc = bacc.Bacc("TRN2", target_bir_lowering=False, debug=False)
    xs = nc.dram_tensor("xs", [128, n_tokens, 128], BF16, kind="ExternalInput")
    u = nc.dram_tensor("u", [128, 128], BF16, kind="ExternalInput")
    v = nc.dram_tensor("v", [128, 128], BF16, kind="ExternalInput")
    ys = nc.dram_tensor("ys", [128, n_tokens, 128], BF16, kind="ExternalOutput")

    with tile.TileContext(nc) as tc:
        with (
            tc.tile_pool(name="const", bufs=1) as cpool,
            tc.tile_pool(name="xin", bufs=sb_bufs) as xpool,
            tc.tile_pool(name="yout", bufs=sb_bufs) as ypool,
            tc.tile_pool(name="pmid", bufs=pipe + 2) as ppool,
            tc.tile_pool(name="psa", bufs=pa_bufs, space="PSUM") as pspool_a,
            tc.tile_pool(name="psb", bufs=pb_bufs, space="PSUM") as pspool_b,
        ):
            u_sb = cpool.tile([128, 128], BF16)
            v_sb = cpool.tile([128, 128], BF16)
            nc.sync.dma_start(u_sb[:], u[:])
            nc.sync.dma_start(v_sb[:], v[:])

            copy_p = nc.scalar.copy if swap_copy else nc.vector.tensor_copy
            copy_y = nc.vector.tensor_copy if swap_copy else nc.scalar.copy
            store_engs = store_eng.split(",")
            load_engs = load_eng.split(",")

            # software pipeline: queue of (psb_tile, yt_tile, q, is_last)
            pending = []
            store_count = [0]

            def mm2_flush():
                psb_t, yt_t, q, store_slice = pending.pop(0)
                pb = pspool_b.tile([128, quad, 128], F32)
                if wide_mm2:
                    for h in range(quad // w4):
                        nc.tensor.matmul(
                            pb[:, h * w4 : (h + 1) * w4, :],
                            lhsT=v_sb[:],
                            rhs=psb_t[:, h * w4 : (h + 1) * w4, :],
                            start=True,
                            stop=True,
                        )
                else:
                    for j in range(quad):
                        nc.tensor.matmul(
                            pb[:, j, :],
                            lhsT=psb_t[:, j, :],
                            rhs=v_sb[:],
                            start=True,
                            stop=True,
                        )
                copy_y(yt_t[:, q * quad : (q + 1) * quad, :], pb[:])
                if store_slice is not None:
                    eng, dst, src_sl = store_slice
                    getattr(nc, eng).dma_start(dst, yt_t[:, src_sl, :])

            # edge_split: loads arrive as [la | group-la] pieces on separate
            # queues (fast first-compute); stores leave as two halves, the
            # first issued mid-group.
            la = 2 * quad if edge_split else group

            n_groups = n_tokens // group
            for g in range(n_groups):
                t0 = g * group
                if first_split and g == 0:
                    # split the first load so compute starts after only a
                    # 2-quad transfer; the rest arrives via the idle Act
                    # queue while quads 0-1 run.
                    fla = 2 * quad
                    xta = xpool.tile([128, fla, 128], BF16, name="xta")
                    nc.sync.dma_start(xta[:], xs[:, t0 : t0 + fla, :])
                    xtb = xpool.tile([128, group - fla, 128], BF16,
                                     name="xtb")
                    nc.scalar.dma_start(
                        xtb[:], xs[:, t0 + fla : t0 + group, :]
                    )
                    la_g = fla
                elif edge_split:
                    xta = xpool.tile([128, la, 128], BF16, name="xta")
                    nc.sync.dma_start(xta[:], xs[:, t0 : t0 + la, :])
                    xtb = xpool.tile([128, group - la, 128], BF16, name="xtb")
                    nc.gpsimd.dma_start(
                        xtb[:], xs[:, t0 + la : t0 + group, :]
                    )
                    la_g = la
                else:
                    xta = xpool.tile([128, group, 128], BF16, name="xt")
                    xtb = xta
                    la_g = group
                    getattr(nc, load_engs[g % len(load_engs)]).dma_start(
                        xta[:], xs[:, t0 : t0 + group, :]
                    )
                yt = ypool.tile([128, group, 128], BF16)
                for q in range(qpg):
                    pa = pspool_a.tile([128, quad, 128], F32)
                    for j in range(quad):
                        t = q * quad + j
                        lhsT = (
                            xta[:, t, :] if t < la_g else xtb[:, t - la_g, :]
                        )
                        nc.tensor.matmul(
                            pa[:, j, :],
                            lhsT=lhsT,
                            rhs=u_sb[:],
                            start=True,
                            stop=True,
                        )
                    psb = ppool.tile([128, quad, 128], BF16)
                    copy_p(psb[:], pa[:])
                    if edge_split or (last_split and g == n_groups - 1):
                        half = group // 2
                        if q == qpg // 2 - 1:
                            store_slice = ("sync", ys[:, t0 : t0 + half, :],
                                           slice(0, half))
                        elif q == qpg - 1:
                            store_slice = ("gpsimd",
                                           ys[:, t0 + half : t0 + group, :],
                                           slice(half, group))
                        else:
                            store_slice = None
                    else:
                        store_slice = (
                            (store_engs[store_count[0] % len(store_engs)],
                             ys[:, t0 : t0 + group, :], slice(0, group))
                            if q == qpg - 1
                            else None
                        )
                    pending.append((psb, yt, q, store_slice))
                    if len(pending) > pipe:
                        mm2_flush()
            while pending:
                mm2_flush()
    nc.compile()
    return nc


_NC_CACHE = {}


def _get_nc(**kw):
    key = tuple(sorted(kw.items()))
    if key not in _NC_CACHE:
        _NC_CACHE[key] = build_nc(**kw)
    return _NC_CACHE[key]


def _prep_inputs(x, U, V):
    """Host-side prep: cast to bf16 and permute x to [i1, T, i2]."""
    x = np.asarray(x, dtype=np.float32)
    U = np.asarray(U, dtype=np.float32)
    V = np.asarray(V, dtype=np.float32)
    t = x.shape[0]
    xs = np.ascontiguousarray(
        x.astype(NP_BF16).reshape(t, 128, 128).transpose(1, 0, 2)
    )
    return xs, U.astype(NP_BF16), V.astype(NP_BF16)


def run(x, U, V, group=32, quad=4, pipe=3, wide_mm2=True, swap_copy=False,
        pa_bufs=3, pb_bufs=3, sb_bufs=4, store_eng="gpsimd,scalar",
        load_eng="sync", trace=False, **spmd_kwargs):
    """Shard over 8 cores, run, gather. Returns (y_full, BassKernelResults)."""
    xs, Ub, Vb = _prep_inputs(x, U, V)
    t_core = xs.shape[1] // N_CORES
    nc = _get_nc(n_tokens=t_core, group=group, quad=quad, pipe=pipe,
                 wide_mm2=wide_mm2, swap_copy=swap_copy, pa_bufs=pa_bufs,
                 pb_bufs=pb_bufs, sb_bufs=sb_bufs, store_eng=store_eng,
                 load_eng=load_eng)
    in_maps = [
        {"xs": np.ascontiguousarray(xs[:, i * t_core : (i + 1) * t_core, :]),
         "u": Ub, "v": Vb}
        for i in range(N_CORES)
    ]
    res = run_bass_kernel_spmd(
        nc, in_maps, list(range(N_CORES)), trace=trace, **spmd_kwargs
    )
    ys = np.stack([res.results[i]["ys"] for i in range(N_CORES)], axis=0)
    if wide_mm2:
        # ys[core] is [j2, t_core, j1] (Y^T tiles) -> y[t, j1*128+j2]
        y = ys.transpose(0, 2, 3, 1).reshape(N_CORES * t_core, D)
    else:
        # ys[core] is [j1, t_core, j2] -> y[t, j1*128+j2]
        y = ys.transpose(0, 2, 1, 3).reshape(N_CORES * t_core, D)
    return y.astype(np.float32), res


def kernel(x, U, V):
    out, _ = run(x, U, V)
    return out


# revision 40
# speedup vs baseline: 1.2081x; 1.2081x over previous
"""Trainium2 Bass kernel for KroneckerLinear: y = x @ kron(U, V).

Math: with x[t] reshaped to X_t [i1=128, i2=128] (i2 contiguous) and
y[t] reshaped to Y_t [j1=128, j2=128] (j2 contiguous):

    Y_t = U^T @ X_t @ V

    MM1 (per token): out = lhsT.T @ rhs, lhsT = X_t [i1, i2] (stationary),
         rhs = U [i1, j1] -> P^T [i2, j1]  (P = U^T X_t)
    MM2 (per quad of 4 tokens, V stationary): lhsT = V [i2, j2],
         rhs = [P^T_t0 | .. | P^T_t3]  [i2, 4*128]
         -> [Y^T_t0 | .. | Y^T_t3]  [j2, 4*128]
    One wide N=512 matmul replaces 4 per-token (LDWEIGHTS + MATMUL) pairs:
    matmul cost scales only with the moving-operand width, and V stays
    semi-stationary (one cheap FWL reload per quad).

Everything runs in bf16 (inputs bf16, PSUM accum fp32, intermediate P and
output y rounded to bf16).  End-to-end rel err vs the fp32 reference is
~4.6e-3 (absmax / max|y|), well under the 2e-2 gate.

DMA layout: x is pre-permuted on the host to xs[i1, t, i2] (bf16) so the
device loads are large fully-contiguous descriptors (G*256 B per partition
per group) instead of the 256 B strided chunks a [t, (i1 i2)] layout would
need (descriptors < 512 B run at half DMA throughput).  y is produced on
device as ys[j2, t, j1] (bf16, Y^T tiles, contiguous stores) and
un-permuted on the host.  This halves HBM traffic vs fp32 while keeping
full descriptor efficiency: ~22 us load + ~22 us store per core.

Engine budget per core (256 tokens, 64 quads):
  PE : per quad 4x(LDW+MM1) + LDW(V) + wide MM2  -> ~480 ns/quad
  DVE: P^T PSUM->SBUF bf16 cast                  -> ~598 ns/quad  <- pace
  Act: Y^T PSUM->SBUF bf16 copy                  -> ~578 ns/quad
  DMA: 16 MB bf16 traffic                        -> ~694 ns/quad eq.
Steady state runs gap-free at ~600 ns/quad (38.4 us); plus ~14 us fixed
startup (NEFF preamble + first load) and ~10 us store/drain tail.

Scheduling details that matter (measured, not theoretical):
  - pipe=3: MM2(q-3) issues after MM1s(q); the in-order PE queue must
    never reach a matmul whose P^T cast hasn't completed, or the PE
    stalls AND drops to a low p-state (first MM after an idle gap runs
    3-4x slow).
  - loads all on the sync (SP) HWDGE queue; stores alternate between
    the gpsimd SWDGE queue and the sync queue.  Each DGE queue admits
    one outstanding DMA (next issue waits prev completion), so a single
    queue cannot sustain loads+stores back-to-back.  Do NOT put stores
    on the Act queue: the DMA issue cost delays the Act-engine Y copies
    (+3 us measured).

Sharding: data-parallel over the token dim, 256 tokens per core x 8 cores.
"""

import sys

if "/opt/trn_rl_repo" not in sys.path:
    sys.path.insert(0, "/opt/trn_rl_repo")

import ml_dtypes
import numpy as np

import concourse.bacc as bacc
import concourse.mybir as mybir
from concourse import tile
from concourse.bass_utils import run_bass_kernel_spmd

F32 = mybir.dt.float32
BF16 = mybir.dt.bfloat16
NP_BF16 = ml_dtypes.bfloat16

N_CORES = 8
TOKENS = 2048
D = 16384  # 128 * 128
T_CORE = TOKENS // N_CORES  # 256


def build_nc(n_tokens=T_CORE, group=32, quad=4, pipe=3, wide_mm2=True,
             swap_copy=False, pa_bufs=3, pb_bufs=3, sb_bufs=4,
             store_eng="gpsimd,sync", load_eng="sync", edge_split=False,
             first_split=False, last_split=False):
    """Build + compile the per-core program.

    group: tokens per DMA transfer (load and store granularity).
    quad:  tokens per PSUM tile / per copy instruction (<=4: a wide MM2
           output of quad*128 fp32 must fit a 2 KB PSUM bank).
    pipe:  software-pipeline distance in quads: MM2(q-pipe) is issued
           after MM1s(q) so the in-order PE queue never reaches a matmul
           whose P^T cast hasn't completed.  0 = no pipelining.
    wide_mm2: one V-stationary N=quad*128 matmul per quad instead of
           per-token (LDWEIGHTS + MATMUL) pairs.
    swap_copy: put the P copy on Act and the Y copy on DVE instead.
    """
    assert n_tokens % group == 0 and group % quad == 0
    qpg = group // quad  # quads per group
    w4 = min(quad, 4)  # tokens per wide MM2 (N=w4*128 <= one PSUM bank)

    nc = bacc.Bacc("TRN2", target_bir_lowering=False, debug=False)
    xs = nc.dram_tensor("xs", [128, n_tokens, 128], BF16, kind="ExternalInput")
    u = nc.dram_tensor("u", [128, 128], BF16, kind="ExternalInput")
    v = nc.dram_tensor("v", [128, 128], BF16, kind="ExternalInput")
    ys = nc.dram_tensor("ys", [128, n_tokens, 128], BF16, kind="ExternalOutput")

    with tile.TileContext(nc) as tc:
        with (
            tc.tile_pool(name="const", bufs=1) as cpool,
            tc.tile_pool(name="xin", bufs=sb_bufs) as xpool,
            tc.tile_pool(name="yout", bufs=sb_bufs) as ypool,
            tc.tile_pool(name="pmid", bufs=pipe + 2) as ppool,
            tc.tile_pool(name="psa", bufs=pa_bufs, space="PSUM") as pspool_a,
            tc.tile_pool(name="psb", bufs=pb_bufs, space="PSUM") as pspool_b,
        ):
            u_sb = cpool.tile([128, 128], BF16)
            v_sb = cpool.tile([128, 128], BF16)
            nc.sync.dma_start(u_sb[:], u[:])
            nc.sync.dma_start(v_sb[:], v[:])

            copy_p = nc.scalar.copy if swap_copy else nc.vector.tensor_copy
            copy_y = nc.vector.tensor_copy if swap_copy else nc.scalar.copy
            store_engs = store_eng.split(",")
            load_engs = load_eng.split(",")

            # software pipeline: queue of (psb_tile, yt_tile, q, is_last)
            pending = []
            store_count = [0]

            def mm2_flush():
                psb_t, yt_t, q, store_slice = pending.pop(0)
                pb = pspool_b.tile([128, quad, 128], F32)
                if wide_mm2:
                    for h in range(quad // w4):
                        nc.tensor.matmul(
                            pb[:, h * w4 : (h + 1) * w4, :],
                            lhsT=v_sb[:],
                            rhs=psb_t[:, h * w4 : (h + 1) * w4, :],
                            start=True,
                            stop=True,
                        )
                else:
                    for j in range(quad):
                        nc.tensor.matmul(
                            pb[:, j, :],
                            lhsT=psb_t[:, j, :],
                            rhs=v_sb[:],
                            start=True,
                            stop=True,
                        )
                copy_y(yt_t[:, q * quad : (q + 1) * quad, :], pb[:])
                if store_slice is not None:
                    eng, dst, src_sl = store_slice
                    getattr(nc, eng).dma_start(dst, yt_t[:, src_sl, :])

            # edge_split: loads arrive as [la | group-la] pieces on separate
            # queues (fast first-compute); stores leave as two halves, the
            # first issued mid-group.
            la = 2 * quad if edge_split else group

            n_groups = n_tokens // group
            for g in range(n_groups):
                t0 = g * group
                if first_split and g == 0:
                    # split the first load so compute starts after only a
                    # 2-quad transfer; the rest arrives via the idle Act
                    # queue while quads 0-1 run.
                    fla = 2 * quad
                    xta = xpool.tile([128, fla, 128], BF16, name="xta")
                    nc.sync.dma_start(xta[:], xs[:, t0 : t0 + fla, :])
                    xtb = xpool.tile([128, group - fla, 128], BF16,
                                     name="xtb")
                    nc.scalar.dma_start(
                        xtb[:], xs[:, t0 + fla : t0 + group, :]
                    )
                    la_g = fla
                elif edge_split:
                    xta = xpool.tile([128, la, 128], BF16, name="xta")
                    nc.sync.dma_start(xta[:], xs[:, t0 : t0 + la, :])
                    xtb = xpool.tile([128, group - la, 128], BF16, name="xtb")
                    nc.gpsimd.dma_start(
                        xtb[:], xs[:, t0 + la : t0 + group, :]
                    )
                    la_g = la
                else:
                    xta = xpool.tile([128, group, 128], BF16, name="xt")
                    xtb = xta
                    la_g = group
                    getattr(nc, load_engs[g % len(load_engs)]).dma_start(
                        xta[:], xs[:, t0 : t0 + group, :]
                    )
                yt = ypool.tile([128, group, 128], BF16)
                for q in range(qpg):
                    pa = pspool_a.tile([128, quad, 128], F32)
                    for j in range(quad):
                        t = q * quad + j
                        lhsT = (
                            xta[:, t, :] if t < la_g else xtb[:, t - la_g, :]
                        )
                        nc.tensor.matmul(
                            pa[:, j, :],
                            lhsT=lhsT,
                            rhs=u_sb[:],
                            start=True,
                            stop=True,
                        )
                    psb = ppool.tile([128, quad, 128], BF16)
                    copy_p(psb[:], pa[:])
                    if edge_split or (last_split and g == n_groups - 1):
                        half = group // 2
                        if q == qpg // 2 - 1:
                            store_slice = ("sync", ys[:, t0 : t0 + half, :],
                                           slice(0, half))
                        elif q == qpg - 1:
                            store_slice = ("gpsimd",
                                           ys[:, t0 + half : t0 + group, :],
                                           slice(half, group))
                        else:
                            store_slice = None
                    else:
                        if q == qpg - 1:
                            eng = store_engs[store_count[0] % len(store_engs)]
                            store_count[0] += 1
                            store_slice = (eng, ys[:, t0 : t0 + group, :],
                                           slice(0, group))
                        else:
                            store_slice = None
                    pending.append((psb, yt, q, store_slice))
                    if len(pending) > pipe:
                        mm2_flush()
            while pending:
                mm2_flush()
    nc.compile()
    return nc


_NC_CACHE = {}


def _get_nc(**kw):
    key = tuple(sorted(kw.items()))
    if key not in _NC_CACHE:
        _NC_CACHE[key] = build_nc(**kw)
    return _NC_CACHE[key]


def _prep_inputs(x, U, V):
    """Host-side prep: cast to bf16 and permute x to [i1, T, i2]."""
    x = np.asarray(x, dtype=np.float32)
    U = np.asarray(U, dtype=np.float32)
    V = np.asarray(V, dtype=np.float32)
    t = x.shape[0]
    xs = np.ascontiguousarray(
        x.astype(NP_BF16).reshape(t, 128, 128).transpose(1, 0, 2)
    )
    return xs, U.astype(NP_BF16), V.astype(NP_BF16)


def run(x, U, V, group=32, quad=4, pipe=3, wide_mm2=True, swap_copy=False,
        pa_bufs=3, pb_bufs=3, sb_bufs=4, store_eng="gpsimd,sync",
        load_eng="sync", first_split=False, last_split=False,
        trace=False, **spmd_kwargs):
    """Shard over 8 cores, run, gather. Returns (y_full, BassKernelResults)."""
    xs, Ub, Vb = _prep_inputs(x, U, V)
    t_core = xs.shape[1] // N_CORES
    nc = _get_nc(n_tokens=t_core, group=group, quad=quad, pipe=pipe,
                 wide_mm2=wide_mm2, swap_copy=swap_copy, pa_bufs=pa_bufs,
                 pb_bufs=pb_bufs, sb_bufs=sb_bufs, store_eng=store_eng,
                 load_eng=load_eng, first_split=first_split,
                 last_split=last_split)
    in_maps = [
        {"xs": np.ascontiguousarray(xs[:, i * t_core : (i + 1) * t_core, :]),
         "u": Ub, "v": Vb}
        for i in range(N_CORES)
    ]
    res = run_bass_kernel_spmd(
        nc, in_maps, list(range(N_CORES)), trace=trace, **spmd_kwargs
    )
    ys = np.stack([res.results[i]["ys"] for i in range(N_CORES)], axis=0)
    if wide_mm2:
        # ys[core] is [j2, t_core, j1] (Y^T tiles) -> y[t, j1*128+j2]
        y = ys.transpose(0, 2, 3, 1).reshape(N_CORES * t_core, D)
    else:
        # ys[core] is [j1, t_core, j2] -> y[t, j1*128+j2]
        y = ys.transpose(0, 2, 1, 3).reshape(N_CORES * t_core, D)
    return y.astype(np.float32), res


def kernel(x, U, V):
    out, _ = run(x, U, V)
    return out
